# revision 18
# baseline (speedup 1.0000x reference)
"""Multi-head forgetting attention on 8 trn2 cores.

Sharding: 2 heads per core (head/tensor parallel). Each core receives the
full (host-pre-transposed) activations, its column slice of Wq/Wk/Wv, its
row slice of Wo^T, and produces a partial (B,S,D) output which the host
sums (+ bo).

Math per core (heads h0, h1 at partitions 0:64 / 64:128):
  QT = (Wq_c/8)^T-proj of x  -> [128, S] (transposed layout, pre-scaled)
  KT likewise (unscaled); V natural [S, 128]
  scoresT[k,q] = KT^T-slice . QT  (fp32r matmuls, PSUM [128,512] tiles)
  gate = sigmoid(gq[q]+gk[k]+gb) = 0.5*tanh(0.5*l) + 0.5
    tanh on ACT (same table set as Exp), fused (t*0.5+0.5)*scores on DVE
    via affine_mul_reduce reading scores straight from PSUM.
  probs = exp(gated) -> bf16; mixed mask blocks multiplied by 0/1 tiles.
  attn-out: lhsT=probs [k,128q], rhs=[V_h|ones] [k,65] -> out[q, 64+rowsum]
  normalize by reciprocal(rowsum) per-partition, PE-transpose [q,128]->[hd,q]
  P_partial = outT^T . WoT_c  -> DMA out fp32.
"""

import os
import sys

sys.path.insert(0, "/opt/trn_rl_repo")

import numpy as np
import ml_dtypes

bf16 = ml_dtypes.bfloat16

B, S, D, H = 2, 2048, 1024, 16
DK = 64
NCORES = 8
HPC = 2          # heads per core
CW = HPC * DK    # 128 per-core head width
P = 128
QTW = 512        # q tile width (matmul free dim)
NQT = S // QTW   # 4
NKT = S // P     # 16 k tiles
NSL = S // P     # 16 q slices
ND = D // P      # 8 contraction tiles

_CACHE = {}


def _prep_mask(mask):
    """Classify 128x128 [q,k] blocks -> 0 skip / 1 full / 2 mixed."""
    m = np.asarray(mask).astype(bool)
    st = np.zeros((B, NKT, NSL), dtype=np.int8)
    m01 = []
    midx = {}
    for b in range(B):
        for i in range(NKT):
            for s in range(NSL):
                blk = m[b, s * P:(s + 1) * P, i * P:(i + 1) * P]
                if blk.all():
                    st[b, i, s] = 1
                elif blk.any():
                    st[b, i, s] = 2
                    midx[(b, i, s)] = len(m01)
                    m01.append(np.ascontiguousarray(blk.T).astype(bf16))
    if not m01:
        m01.append(np.zeros((P, P), dtype=bf16))
    return st, np.stack(m01), midx


def _build(meta):
    """Build the (shared-across-cores) bass program."""
    import concourse.bass as bass
    import concourse.mybir as mybir
    import concourse.tile as tile
    from concourse import bacc

    st = meta["st"]
    midx = meta["midx"]
    n_m01 = meta["n_m01"]
    gb = meta["gb"]
    use_bq = meta["use_bq"]
    use_bk = meta["use_bk"]
    use_bv = meta["use_bv"]

    f32 = mybir.dt.float32
    f32r = mybir.dt.float32r
    b16 = mybir.dt.bfloat16
    Act = mybir.ActivationFunctionType

    # block tables
    valid_qt = {}
    qt0 = {}
    qt1 = {}
    for b in range(B):
        for i in range(NKT):
            v = [qt for qt in range(NQT)
                 if any(st[b, i, 4 * qt + j] for j in range(4))]
            valid_qt[(b, i)] = v
            if v:
                qt0[(b, i)] = v[0]
                qt1[(b, i)] = v[-1] + 1
    valid_i = {(b, s): [i for i in range(NKT) if st[b, i, s]]
               for b in range(B) for s in range(NSL)}

    nc = bacc.Bacc("TRN2", debug=False, enable_asserts=False,
                   num_devices=NCORES)

    xqt = nc.dram_tensor("xqt", (B, D, S), b16, kind="ExternalInput")
    xkt = nc.dram_tensor("xkt", (B, D, S), b16, kind="ExternalInput")
    xvt = nc.dram_tensor("xvt", (B, D, S), b16, kind="ExternalInput")
    wqt = nc.dram_tensor("wqt", (P, D), b16, kind="ExternalInput")
    wkt = nc.dram_tensor("wkt", (P, D), b16, kind="ExternalInput")
    wvt = nc.dram_tensor("wvt", (P, D), b16, kind="ExternalInput")
    wot = nc.dram_tensor("wot", (P, D), b16, kind="ExternalInput")
    wgq = nc.dram_tensor("wgq", (P, DK + 1), f32r, kind="ExternalInput")
    wgk = nc.dram_tensor("wgk", (P, HPC), f32, kind="ExternalInput")
    identd = nc.dram_tensor("identd", (P, P), b16, kind="ExternalInput")
    onesd = nc.dram_tensor("onesd", (DK + 1, P), f32r, kind="ExternalInput")
    m01d = nc.dram_tensor("m01d", (n_m01, P, P), b16, kind="ExternalInput")
    bqt = nc.dram_tensor("bqt", (P, 1), f32, kind="ExternalInput")
    bkt = nc.dram_tensor("bkt", (P, 1), f32, kind="ExternalInput")
    bvt = nc.dram_tensor("bvt", (P, P), f32, kind="ExternalInput")
    outp = nc.dram_tensor("outp", (B, S, D), f32, kind="ExternalOutput")

    with tile.TileContext(nc) as tc:
        from contextlib import ExitStack
        with ExitStack() as ctx:
            cst = ctx.enter_context(tc.tile_pool(name="cst", bufs=1))
            per = ctx.enter_context(tc.tile_pool(name="per", bufs=1))
            strm = ctx.enter_context(tc.tile_pool(name="strm", bufs=3))
            work = ctx.enter_context(tc.tile_pool(name="work", bufs=2))
            prb = ctx.enter_context(tc.tile_pool(name="prb", bufs=1))
            mis = ctx.enter_context(
                tc.tile_pool(name="mis", bufs=2, space="PSUM"))
            scp = ctx.enter_context(
                tc.tile_pool(name="scp", bufs=2, space="PSUM"))
            att = ctx.enter_context(
                tc.tile_pool(name="att", bufs=1, space="PSUM"))

            # ---- constants ----
            wq_sb = cst.tile([P, D], b16, name="wq_sb")
            wk_sb = cst.tile([P, D], b16, name="wk_sb")
            wv_sb = cst.tile([P, D], b16, name="wv_sb")
            wo_sb = cst.tile([P, D], b16, name="wo_sb")
            wgq_sb = cst.tile([P, DK + 1], f32r, name="wgq_sb")
            wgk_sb = cst.tile([P, HPC], f32, name="wgk_sb")
            id_sb = cst.tile([P, P], b16, name="id_sb")
            ones_sb = cst.tile([DK + 1, P], f32r, name="ones_sb")
            bq_sb = cst.tile([P, 1], f32, name="bq_sb")
            bk_sb = cst.tile([P, 1], f32, name="bk_sb")
            bv_sb = cst.tile([P, P], f32, name="bv_sb")
            nc.sync.dma_start(wq_sb[:], wqt[:, :])
            nc.sync.dma_start(wk_sb[:], wkt[:, :])
            nc.sync.dma_start(wv_sb[:], wvt[:, :])
            nc.sync.dma_start(wo_sb[:], wot[:, :])
            nc.sync.dma_start(wgq_sb[:], wgq[:, :])
            nc.sync.dma_start(wgk_sb[:], wgk[:, :])
            nc.sync.dma_start(id_sb[:], identd[:, :])
            nc.sync.dma_start(ones_sb[:], onesd[:, :])
            if use_bq:
                nc.sync.dma_start(bq_sb[:], bqt[:, :])
            if use_bk:
                nc.sync.dma_start(bk_sb[:], bkt[:, :])
            if use_bv:
                nc.sync.dma_start(bv_sb[:], bvt[:, :])

            # persistent per-b buffers (reused across b)
            qt_sb = per.tile([P, S], f32r, name="qt_sb")
            kt_sb = per.tile([P, S], f32r, name="kt_sb")
            # V with a ones column every 65: per head
            v_sb = [per.tile([P, NKT * (DK + 1)], b16, name=f"v{h}_sb")
                    for h in range(HPC)]
            gq_sb = per.tile([DK + 1, S], f32r, name="gq_sb")
            gk_sb = [per.tile([P, NKT], f32, name=f"gk{h}_sb")
                     for h in range(HPC)]
            gqb_sb = [per.tile([P, S], b16, name=f"gqb{h}_sb")
                      for h in range(HPC)]
            outt_sb = per.tile([P, S], b16, name="outt_sb")

            # set the ones columns of V once
            for h in range(HPC):
                nc.vector.memset(
                    v_sb[h][:, DK::DK + 1], 1.0)

            for b in range(B):
                # ---------- V projection (xv streamed per 512-chunk) ----
                for qt in range(NQT):
                    xv = []
                    for dt in range(ND):
                        t = strm.tile([P, QTW], b16, name=f"xv{dt}",
                                      tag=f"xv{dt}", bufs=2)
                        nc.sync.dma_start(
                            t[:], xvt[b, dt * P:(dt + 1) * P,
                                      qt * QTW:(qt + 1) * QTW])
                        xv.append(t)
                    for sj in range(QTW // P):
                        sl = qt * (QTW // P) + sj
                        vps = mis.tile([P, P], f32, tag="mis", name="vps")
                        for dt in range(ND):
                            nc.tensor.matmul(
                                vps[:],
                                lhsT=xv[dt][:, sj * P:(sj + 1) * P],
                                rhs=wv_sb[:, dt * P:(dt + 1) * P],
                                start=(dt == 0), stop=(dt == ND - 1))
                        if use_bv:
                            nc.vector.tensor_add(vps[:], vps[:], bv_sb[:])
                        for h in range(HPC):
                            nc.vector.tensor_copy(
                                v_sb[h][:, sl * (DK + 1):sl * (DK + 1) + DK],
                                vps[:, h * DK:(h + 1) * DK])

                # ---------- Q/K transposed projections ----------
                for (xsrc, wsb, osb, bias_sb, use_b) in (
                        (xqt, wq_sb, qt_sb, bq_sb, use_bq),
                        (xkt, wk_sb, kt_sb, bk_sb, use_bk)):
                    for qt in range(NQT):
                        pps = mis.tile([P, QTW], f32, tag="mis", name="pps")
                        for dt in range(ND):
                            xt = strm.tile([P, QTW], b16, tag="xt",
                                           name="xt")
                            nc.sync.dma_start(
                                xt[:],
                                xsrc[b, dt * P:(dt + 1) * P,
                                     qt * QTW:(qt + 1) * QTW])
                            nc.tensor.matmul(
                                pps[:],
                                lhsT=wsb[:, dt * P:(dt + 1) * P],
                                rhs=xt[:],
                                start=(dt == 0), stop=(dt == ND - 1))
                        dst = osb[:, qt * QTW:(qt + 1) * QTW]
                        if use_b:
                            nc.scalar.activation(
                                dst, pps[:], Act.Identity, bias=bias_sb[:])
                        else:
                            nc.vector.tensor_copy(dst, pps[:])

                # ---------- gate vectors ----------
                for qt in range(NQT):
                    gps = mis.tile([DK + 1, QTW], f32, tag="mis", name="gps")
                    nc.tensor.matmul(
                        gps[:],
                        lhsT=wgq_sb[:],
                        rhs=qt_sb[:, qt * QTW:(qt + 1) * QTW],
                        start=True, stop=True)
                    nc.vector.tensor_copy(
                        gq_sb[:, qt * QTW:(qt + 1) * QTW], gps[:])
                for h in range(HPC):
                    hsl = slice(h * DK, (h + 1) * DK)
                    gkp = mis.tile([P, NKT], f32, tag="mis", name="gkp")
                    for i in range(NKT):
                        nc.tensor.matmul(
                            gkp[:, i:i + 1],
                            lhsT=kt_sb[hsl, i * P:(i + 1) * P].bitcast(f32),
                            rhs=wgk_sb[hsl, h:h + 1],
                            start=(i == 0), stop=(i == NKT - 1),
                            skip_group_check=True)
                    # gk05 = 0.5*gk + 0.5*gb
                    nc.scalar.activation(
                        gk_sb[h][:], gkp[:], Act.Identity,
                        bias=0.5 * gb, scale=0.5)

                # ---------- gq broadcast [k-part, q] ----------
                for h in range(HPC):
                    for qt in range(NQT):
                        gbp = mis.tile([P, QTW], f32, tag="mis", name="gbp")
                        nc.tensor.matmul(
                            gbp[:],
                            lhsT=ones_sb[h * DK:h * DK + 1, :],
                            rhs=gq_sb[h * DK:h * DK + 1,
                                      qt * QTW:(qt + 1) * QTW],
                            start=True, stop=True)
                        nc.vector.tensor_copy(
                            gqb_sb[h][:, qt * QTW:(qt + 1) * QTW], gbp[:])

                # ---------- attention: probs ----------
                probs = {}
                for i in range(NKT):
                    if (b, i) not in qt0:
                        continue
                    lo = qt0[(b, i)]
                    hi = qt1[(b, i)]
                    w = (hi - lo) * QTW
                    tanh_t = {}
                    for h in range(HPC):
                        t = work.tile([P, w], f32, tag=f"tanh{h}",
                                      name=f"tanh{h}", bufs=1)
                        nc.scalar.activation(
                            t[:], gqb_sb[h][:, lo * QTW:hi * QTW],
                            Act.Tanh, bias=gk_sb[h][:, i:i + 1], scale=0.5)
                        tanh_t[h] = t
                    gat = {}
                    for h in range(HPC):
                        gat[h] = work.tile([P, w], f32, tag=f"gat{h}",
                                           name=f"gat{h}", bufs=1)
                    for qt in range(lo, hi):
                        scps = {}
                        for h in range(HPC):
                            hsl = slice(h * DK, (h + 1) * DK)
                            sp = scp.tile([P, QTW], f32, tag=f"sc{h}",
                                          name=f"sc{h}")
                            nc.tensor.matmul(
                                sp[:],
                                lhsT=kt_sb[hsl,
                                           i * P:(i + 1) * P],
                                rhs=qt_sb[hsl,
                                          qt * QTW:(qt + 1) * QTW
                                          ],
                                start=True, stop=True)
                            scps[h] = sp
                        for h in range(HPC):
                            acc = work.tile([P, 1], f32, tag="acc",
                                            name="acc")
                            o = (qt - lo) * QTW
                            nc.vector.affine_mul_reduce(
                                gat[h][:, o:o + QTW], acc[:],
                                tanh_t[h][:, o:o + QTW], scps[h][:],
                                0.5, 0.5)
                    for h in range(HPC):
                        pt = prb.tile([P, w], b16, tag=f"pr{h}_{i}",
                                      name=f"pr{h}_{i}")
                        nc.scalar.activation(pt[:], gat[h][:], Act.Exp)
                        probs[(h, i)] = pt
                    # mixed-block masking
                    for s in range(lo * 4, hi * 4):
                        if st[b, i, s] == 2:
                            mt = strm.tile([P, P], b16, tag="m01",
                                           name="mt")
                            nc.sync.dma_start(
                                mt[:], m01d[midx[(b, i, s)], :, :])
                            for h in range(HPC):
                                sl2 = slice(s * P - qt0[(b, i)] * QTW,
                                            (s + 1) * P - qt0[(b, i)] * QTW)
                                nc.vector.tensor_mul(
                                    probs[(h, i)][:, sl2],
                                    probs[(h, i)][:, sl2], mt[:])

                # ---------- attn @ V, normalize, transpose ----------
                for s in range(NSL):
                    onat = work.tile([P, P], b16, tag="onat", name="onat")
                    for h in range(HPC):
                        ops = att.tile([P, DK + 1], f32, tag=f"o{h}",
                                       name=f"o{h}")
                        vi = valid_i[(b, s)]
                        if not vi:
                            nc.vector.memset(ops[:], 0.0)
                        for n, i in enumerate(vi):
                            o = s * P - qt0[(b, i)] * QTW
                            nc.tensor.matmul(
                                ops[:],
                                lhsT=probs[(h, i)][:, o:o + P],
                                rhs=v_sb[h][:, i * (DK + 1):
                                            (i + 1) * (DK + 1)],
                                start=(n == 0), stop=(n == len(vi) - 1))
                        rsum = work.tile([P, 1], f32, tag="rsum",
                                         name="rsum")
                        nc.vector.tensor_scalar_add(
                            rsum[:], ops[:, DK:DK + 1], 1e-30)
                        recip = work.tile([P, 1], f32, tag="recip",
                                          name="recip")
                        nc.vector.reciprocal_approx_fast(recip[:], rsum[:])
                        nc.vector.tensor_scalar_mul(
                            onat[:, h * DK:(h + 1) * DK],
                            ops[:, 0:DK], recip[:])
                    trp = mis.tile([P, P], b16, tag="mis", name="trp")
                    nc.tensor.transpose(trp[:], onat[:], id_sb[:])
                    nc.vector.tensor_copy(
                        outt_sb[:, s * P:(s + 1) * P], trp[:])

                # ---------- output projection ----------
                for sl in range(NSL):
                    for nt in range(2):
                        pps = mis.tile([P, QTW], f32, tag="mis", name="fps")
                        nc.tensor.matmul(
                            pps[:],
                            lhsT=outt_sb[:, sl * P:(sl + 1) * P],
                            rhs=wo_sb[:, nt * QTW:(nt + 1) * QTW],
                            start=True, stop=True)
                        po = work.tile([P, QTW], f32, tag="po", name="po")
                        if nt == 0:
                            nc.vector.tensor_copy(po[:], pps[:])
                        else:
                            nc.scalar.copy(po[:], pps[:])
                        nc.sync.dma_start(
                            outp[b, sl * P:(sl + 1) * P,
                                 nt * QTW:(nt + 1) * QTW], po[:])
    nc.compile()
    return nc


def _host_prep(inputs):
    q = np.asarray(inputs["query"], np.float32)
    k = np.asarray(inputs["key"], np.float32)
    v = np.asarray(inputs["value"], np.float32)
    mask = np.asarray(inputs["mask"])
    Wq = np.asarray(inputs["Wq"], np.float32)
    Wk = np.asarray(inputs["Wk"], np.float32)
    Wv = np.asarray(inputs["Wv"], np.float32)
    Wo = np.asarray(inputs["Wo"], np.float32)
    bq = np.asarray(inputs["bq"], np.float32)
    bk = np.asarray(inputs["bk"], np.float32)
    bv = np.asarray(inputs["bv"], np.float32)
    bo = np.asarray(inputs["bo"], np.float32)
    wgq = np.asarray(inputs["wgq"], np.float32)
    wgk = np.asarray(inputs["wgk"], np.float32)
    gb = float(np.asarray(inputs["gb"]))

    st, m01, midx = _prep_mask(mask)

    xqt = np.ascontiguousarray(q.transpose(0, 2, 1)).astype(bf16)
    xkt = np.ascontiguousarray(k.transpose(0, 2, 1)).astype(bf16)
    xvt = np.ascontiguousarray(v.transpose(0, 2, 1)).astype(bf16)

    def wslice(W, c, scale=1.0):
        # W.T column slice [D, 128] -> [128, 8, 128] -> [128, 1024]
        wt = (W.T[:, c * CW:(c + 1) * CW] * scale).astype(bf16)
        return np.ascontiguousarray(
            wt.reshape(ND, P, CW).transpose(1, 0, 2).reshape(P, D))

    scale = 1.0 / np.sqrt(DK)
    ident = np.eye(P, dtype=bf16)

    meta = {
        "st": st, "midx": midx, "n_m01": len(m01), "gb": gb,
        "use_bq": bool(np.any(bq)), "use_bk": bool(np.any(bk)),
        "use_bv": bool(np.any(bv)),
    }

    # gq rows land at partitions 0 and 64 (base-partition constraint)
    wgq_bd = np.zeros((P, DK + 1), np.float32)
    wgk_bd = np.zeros((P, HPC), np.float32)
    for h in range(HPC):
        wgq_bd[h * DK:(h + 1) * DK, h * DK] = wgq / scale
        wgk_bd[h * DK:(h + 1) * DK, h] = wgk

    in_maps = []
    for c in range(NCORES):
        im = {
            "xqt": xqt, "xkt": xkt, "xvt": xvt,
            "wqt": wslice(Wq, c, scale),
            "wkt": wslice(Wk, c),
            "wvt": wslice(Wv, c),
            "wot": np.ascontiguousarray(
                Wo.T[c * CW:(c + 1) * CW, :]).astype(bf16),
            "wgq": wgq_bd, "wgk": wgk_bd,
            "identd": ident, "m01d": m01,
            "onesd": np.ones((DK + 1, P), np.float32),
            "bqt": (bq[c * CW:(c + 1) * CW] * scale
                    ).reshape(P, 1).astype(np.float32),
            "bkt": bk[c * CW:(c + 1) * CW].reshape(P, 1).astype(np.float32),
            "bvt": np.tile(bv[c * CW:(c + 1) * CW], (P, 1)
                           ).astype(np.float32),
        }
        in_maps.append(im)
    return meta, in_maps, bo


def kernel(**inputs):
    meta, in_maps, bo = _host_prep(inputs)

    key = (meta["st"].tobytes(), meta["gb"], meta["use_bq"],
           meta["use_bk"], meta["use_bv"], meta["n_m01"])
    if key not in _CACHE:
        _CACHE[key] = _build(meta)
    nc = _CACHE[key]

    from concourse.bass_utils import run_bass_kernel_spmd
    res = run_bass_kernel_spmd(
        nc, in_maps, core_ids=list(range(NCORES)),
        trace=bool(int(os.environ.get("KERNEL_TRACE", "0"))))
    out = np.zeros((B, S, D), np.float32)
    for r in res.results:
        out += r["outp"].astype(np.float32)
    out += bo
    if res.exec_time_ns is not None:
        print(f"HW exec time: {res.exec_time_ns} ns")
    return out


if __name__ == "__main__":
    rng = np.random.default_rng(0)
    print("smoke build only")
